# revision 1
# baseline (speedup 1.0000x reference)
"""Trainium2 Bass kernel for nn_Attention_49168785605257.

Causal multi-head self-attention: B=2, N=4096, DIM=512, H=8, DH=64.
Reference applies dim_head**-0.5 scaling TWICE (folded here into Wq as 1/64).

Sharding: one head per NeuronCore (8 cores). Each core computes its head's
attention for BOTH batches (packed into partition halves 0-63 / 64-127) and
its partial output projection o_h = attn_out_h @ Wo[64h:64h+64, :].  The host
sums the 8 partials and adds the bias.

Device-side formulation (per core):
  - All tensors carried transposed ([feature, token]) so the contraction dim
    sits on SBUF partitions; the host pre-transposes x.
  - Flash-attention in S^T orientation: S^T[j,i] tiles come straight out of
    the PE; exp on ScalarE (PSUM->SBUF, [128,1024] groups covering both
    batches); causal masking by multiplying the exp output of the 4 diagonal
    j-blocks per i-chunk with precomputed 0/1 masks; A@V accumulated in PSUM
    with v augmented by a ones-column so row 64 collects the softmax
    denominators; 1/den via Ln then Exp(-x) (one ACT table set); denominator
    broadcast across partitions on GPSIMD; normalize late (softmax linearity).
"""

import os
import sys
from contextlib import ExitStack

import numpy as np

for _p in ("/opt/trn_rl_repo", "/root/.axon_site/_ro/trn_rl_repo"):
    if _p not in sys.path and os.path.isdir(_p):
        sys.path.append(_p)

import ml_dtypes  # noqa: E402

B, N, DIM, H, DH = 2, 4096, 512, 8, 64
N_CORES = 8
CH = 512            # i-chunk width (tokens)
JB = 128            # j-block width (tokens)

BF16 = "bfloat16"
F32 = "float32"


def _pin_act_tables():
    """Make Exp and Ln resolve only to the natural_log_exp set so the kernel
    never swaps ACT table sets (each swap costs ~2.7us and we interleave
    exp-softmax with ln/exp reciprocals). Best-effort: on any surprise just
    leave the default table selection in place (slower, still correct)."""
    try:
        import concourse.bacc as bacc
        import concourse.hw_specs as hw_specs
        import concourse.mybir as mybir
        orig = hw_specs.get_activation_tables

        def patched(module_arch):
            try:
                tabs = dict(orig(module_arch))
                both = {mybir.ActivationFunctionType.Exp, mybir.ActivationFunctionType.Ln}
                target = None
                for name, funcs in tabs.items():
                    if both <= funcs:
                        target = name
                        break
                if target is None:
                    return tabs
                out = {}
                for name, funcs in tabs.items():
                    out[name] = set(funcs) if name == target else set(funcs) - both
                return out
            except Exception:
                return orig(module_arch)

        bacc.get_activation_tables = patched
    except Exception:
        pass


def build_attention_kernel(nc, NB: int):
    """Emit the per-core program. NB = tokens per batch (4096 full size)."""
    import concourse.mybir as mybir
    import concourse.tile as tile
    _pin_act_tables()

    bf16 = mybir.dt.bfloat16
    f32 = mybir.dt.float32
    mult = mybir.AluOpType.mult
    Exp = mybir.ActivationFunctionType.Exp
    Ln = mybir.ActivationFunctionType.Ln

    NCH = NB // CH          # i-chunks per batch
    JTB = NB // JB          # j-blocks per batch

    xT_d = nc.dram_tensor("xT", [DIM, 2 * NB], bf16, kind="ExternalInput").ap()
    wq_d = nc.dram_tensor("wq", [128, 4 * DH], bf16, kind="ExternalInput").ap()
    wk_d = nc.dram_tensor("wk", [128, 4 * DH], bf16, kind="ExternalInput").ap()
    wv_d = nc.dram_tensor("wv", [128, 4 * DH], bf16, kind="ExternalInput").ap()
    wo_d = nc.dram_tensor("wo", [DH, DIM], bf16, kind="ExternalInput").ap()
    mask_d = nc.dram_tensor("masks", [128, 4096], bf16, kind="ExternalInput").ap()
    idup_d = nc.dram_tensor("identup", [128, DH], bf16, kind="ExternalInput").ap()
    oT_d = nc.dram_tensor("oT", [DIM, 2 * NB], bf16, kind="ExternalOutput").ap()

    with tile.TileContext(nc) as tc, ExitStack() as ctx:
        const = ctx.enter_context(tc.tile_pool(name="const", bufs=1))
        xpool = ctx.enter_context(tc.tile_pool(name="xp", bufs=16))
        big = ctx.enter_context(tc.tile_pool(name="big", bufs=1))
        ptp = ctx.enter_context(tc.tile_pool(name="ptp", bufs=8))
        rp = ctx.enter_context(tc.tile_pool(name="rp", bufs=3))
        op_sb_pool = ctx.enter_context(tc.tile_pool(name="osb", bufs=4))
        ps_pool = ctx.enter_context(tc.tile_pool(name="ps", bufs=2, space="PSUM"))
        av_pool = ctx.enter_context(tc.tile_pool(name="av", bufs=1, space="PSUM"))
        pv_pool = ctx.enter_context(tc.tile_pool(name="pv", bufs=2, space="PSUM"))

        # ---- weights first so chunk-0 projections can start ASAP ----
        wq_sb = const.tile([128, 4 * DH], bf16, tag="wq")
        wk_sb = const.tile([128, 4 * DH], bf16, tag="wk")
        wv_sb = const.tile([128, 4 * DH], bf16, tag="wv")
        nc.sync.dma_start(wq_sb[:], wq_d[:, :])
        nc.sync.dma_start(wk_sb[:], wk_d[:, :])
        nc.sync.dma_start(wv_sb[:], wv_d[:, :])
        wo_sb = const.tile([DH, DIM], bf16, tag="wo")
        mask_sb = const.tile([128, 4096], bf16, tag="mask")
        idup_sb = const.tile([128, DH], bf16, tag="idup")

        # ---- persistent activations (partition halves: rows 0-63 batch0, 64-127 batch1) ----
        qT = big.tile([128, NB], bf16, tag="qT")
        kT = big.tile([128, NB], bf16, tag="kT")
        vT = big.tile([128, NB], bf16, tag="vT")
        vaug = [big.tile([128, 65 * JTB], bf16, tag=f"vaug{b}", name=f"vaug{b}")
                for b in range(2)]

        xts_pend = {}

        def emit_xt(c):
            """Issue the x-chunk DMAs for chunk c (early, to dodge sync-queue
            head-of-line blocking behind epilogue output DMAs)."""
            xts = []
            for d in range(4):
                xt = xpool.tile([128, 1024], bf16, tag="xt", name=f"xt{c}_{d}")
                nc.gpsimd.dma_start(xt[:], xT_d[128 * d:128 * (d + 1), 1024 * c:1024 * (c + 1)])
                xts.append(xt)
            xts_pend[c] = xts

        def emit_chunk_prep(c, ws=(0, 1, 2)):
            """q/k/v projections (+ v transposes) for chunk c."""
            i0 = CH * c
            if 2 in ws:
                xts = xts_pend.pop(c)
            else:
                xts = xts_pend[c]
            if c == 0:
                nc.sync.dma_start(idup_sb[:], idup_d[:, :])
            all_w = ((wq_sb, qT), (wk_sb, kT), (wv_sb, vT))
            for w_sb, dst in (all_w[i] for i in ws):
                ps = pv_pool.tile([128, CH], f32, tag="pv")
                for d in range(4):
                    nc.tensor.matmul(ps[0:64, :], w_sb[:, d * DH:(d + 1) * DH], xts[d][:, 0:512],
                                     start=(d == 0), stop=(d == 3), tile_position=(0, 0),
                                     skip_group_check=True)
                    nc.tensor.matmul(ps[64:128, :], w_sb[:, d * DH:(d + 1) * DH], xts[d][:, 512:1024],
                                     start=(d == 0), stop=(d == 3), tile_position=(0, 64),
                                     skip_group_check=True)
                nc.vector.tensor_copy(dst[:, i0:i0 + CH], ps[:, :])
            if c == 0:
                nc.sync.dma_start(mask_sb[:], mask_d[:, :])
                nc.sync.dma_start(wo_sb[:], wo_d[:, :])
            if 2 not in ws:
                return
            for tt in range(4 * c, 4 * c + 4):
                pst0 = pv_pool.tile([128, 64], bf16, tag="pv", name="pst0")
                pst1 = pv_pool.tile([128, 64], bf16, tag="pv", name="pst1")
                nc.tensor.matmul(pst0[:], vT[0:64, JB * tt:JB * (tt + 1)], idup_sb[0:64, :],
                                 is_transpose=True, tile_position=(0, 0), skip_group_check=True)
                nc.tensor.matmul(pst1[:], vT[64:128, JB * tt:JB * (tt + 1)], idup_sb[64:128, :],
                                 is_transpose=True, tile_position=(64, 0), skip_group_check=True)
                nc.vector.tensor_copy(vaug[0][:, 65 * tt:65 * tt + 64], pst0[:])
                nc.vector.tensor_copy(vaug[1][:, 65 * tt:65 * tt + 64], pst1[:])

        def emit_epilogue_a2(c, outT_un):
            """1/den chain, off the attention critical path.  DVE reciprocal
            (same-partition: engines cannot shift partitions) keeps ScalarE —
            the exp bottleneck — free; the DMA bounce moves the row to
            partition 0 for the GPSIMD broadcast."""
            denp0 = rp.tile([1, 1024], f32, tag="denp0")
            nc.gpsimd.dma_start(denp0[0:1, :], outT_un[64:65, 0:1024])
            recip = rp.tile([1, 1024], f32, tag="recip")
            nc.vector.reciprocal_approx_fast(recip[0:1, :], denp0[0:1, :])
            recipb = rp.tile([64, 1024], f32, tag="recipb")
            nc.gpsimd.partition_broadcast(recipb[0:64, :], recip[0:1, :], channels=64)
            return recipb

        def emit_epilogue_b(c, outT_un, recipb, dblks=range(4), outTn=None):
            """Deferred per-chunk tail: normalize and project; stream out."""
            if outTn is None:
                outTn = rp.tile([64, 1024], bf16, tag="outTn")
                nc.vector.tensor_tensor(outTn[:], outT_un[0:64, 0:1024], recipb[:], mult)
            for dblk in dblks:
                o_sb = op_sb_pool.tile([128, 1024], bf16, tag="o")
                for b in range(2):
                    opp = pv_pool.tile([128, 512], f32, tag="pv", name=f"opp{b}")
                    nc.tensor.matmul(opp[:], wo_sb[:, 128 * dblk:128 * (dblk + 1)],
                                     outTn[0:64, 512 * b:512 * b + 512],
                                     skip_group_check=True)
                    nc.vector.tensor_copy(o_sb[:, 512 * b:512 * (b + 1)], opp[:])
                nc.sync.dma_start(oT_d[128 * dblk:128 * (dblk + 1), 1024 * c:1024 * (c + 1)],
                                  o_sb[:])
            return outTn

        n_up = min(2, NCH)          # chunks prepped upfront (short early chunks)
        emit_xt(0)
        nc.gpsimd.memset(vaug[0][:], 1.0)
        nc.gpsimd.memset(vaug[1][:], 1.0)
        for cc in range(1, n_up):
            emit_xt(cc)
        for cc in range(n_up):
            emit_chunk_prep(cc)
        if n_up < NCH:
            emit_xt(n_up)
        pending_b = None
        for c in range(NCH):
            i0 = CH * c
            # ---- attention for i-chunk c (prev tail + next prep interleaved) ----
            pso = av_pool.tile([65, 1024], f32, tag="av")
            njb = 4 * (c + 1)
            mid = max(1, njb // 2)
            for jb in range(njb):
                if jb == 2 and pending_b is not None:
                    pending_b = pending_b[:3] + (emit_epilogue_b(*pending_b, dblks=(0, 1)),)
                    if n_up <= c + 2 < NCH:
                        emit_xt(c + 2)
                if jb == 4 and pending_b is not None and len(pending_b) == 4:
                    emit_epilogue_b(*pending_b[:3], dblks=(2, 3), outTn=pending_b[3])
                    pending_b = None
                if jb == mid and n_up <= c + 1 < NCH:
                    emit_chunk_prep(c + 1, ws=(0, 1))
                if jb == mid + 2 and n_up <= c + 1 < NCH:
                    emit_chunk_prep(c + 1, ws=(2,))
                # diagonal blocks: only i-columns >= 128t are causally valid
                t = jb - 4 * c
                off = 128 * t if t > 0 else 0
                w = CH - off
                pss = ps_pool.tile([128, 1024], f32, tag="s")
                nc.tensor.matmul(pss[:, off:512], kT[0:64, JB * jb:JB * (jb + 1)],
                                 qT[0:64, i0 + off:i0 + CH],
                                 start=True, stop=True, tile_position=(0, 0), skip_group_check=True)
                nc.tensor.matmul(pss[:, 512 + off:1024], kT[64:128, JB * jb:JB * (jb + 1)],
                                 qT[64:128, i0 + off:i0 + CH],
                                 start=True, stop=True, tile_position=(64, 0), skip_group_check=True)
                pt = ptp.tile([128, 1024], bf16, tag="pt")
                if off:
                    sub = lambda ap: ap.rearrange("p (h w) -> p h w", h=2)[:, :, off:]
                    nc.scalar.activation(sub(pt[:]), sub(pss[:]), Exp)
                else:
                    nc.scalar.activation(pt[:], pss[:], Exp)
                if t >= 0:
                    # only the 128-col diagonal square is partially masked
                    msub = pt[:].rearrange("p (h w) -> p h w", h=2)[:, :, off:off + 128]
                    nc.vector.tensor_tensor(
                        msub, msub,
                        mask_sb[:, 0:256].rearrange("p (h w) -> p h w", h=2), mult)
                nc.tensor.matmul(pso[:, off:512], vaug[0][:, 65 * jb:65 * jb + 65], pt[:, off:512],
                                 start=(jb == 0), stop=(jb == njb - 1), skip_group_check=True)
                nc.tensor.matmul(pso[:, 512 + off:1024], vaug[1][:, 65 * jb:65 * jb + 65],
                                 pt[:, 512 + off:1024],
                                 start=(jb == 0), stop=(jb == njb - 1), skip_group_check=True)

            # ---- epilogue part A: evacuate pso in one fp32 copy ----
            outT_un = rp.tile([65, 1024], f32, tag="outT_un")
            nc.vector.tensor_copy(outT_un[:], pso[0:65, 0:1024])
            if (c + 2) < NCH and (c + 2) not in xts_pend and n_up <= c + 2:
                emit_xt(c + 2)   # fallback if the jb==2 site did not fire
            pending_b = (c, outT_un, emit_epilogue_a2(c, outT_un))
        if pending_b is not None:
            emit_epilogue_b(*pending_b)
    return nc


def make_host_constants(NB: int):
    """Masks for the 4 diagonal j-block offsets and the stacked identity."""
    jj = np.arange(JB)[:, None]
    ii = np.arange(JB)[None, :]
    masks = np.zeros((128, 4096), np.float32)
    m = (ii >= jj).astype(np.float32)                    # [128, 128] diagonal square
    masks[:, 0:256] = np.concatenate([m, m], axis=1)
    identup = np.concatenate([np.eye(DH, dtype=np.float32)] * 2, axis=0)  # [128, 64]
    return (masks.astype(ml_dtypes.bfloat16), identup.astype(ml_dtypes.bfloat16))


_CACHE = {}


def _get_compiled(NB: int):
    key = ("nc", NB)
    if key not in _CACHE:
        import concourse.bacc as bacc
        nc = bacc.Bacc("TRN2", debug=False, num_devices=N_CORES)
        build_attention_kernel(nc, NB)
        nc.compile()
        _CACHE[key] = nc
    return _CACHE[key]


def make_in_maps(x, Wq, Wkv, Wo, NB: int):
    bf = ml_dtypes.bfloat16
    NB = x.shape[1]
    nb_total = x.shape[0] * NB
    xT = x.reshape(nb_total, DIM).T            # [512, B*NB], batch-major cols
    xT = xT.reshape(DIM, 2, NB // CH, CH).transpose(0, 2, 1, 3).reshape(DIM, nb_total)
    xT = np.ascontiguousarray(xT).astype(bf)   # chunk-paired: col = 1024c + 512b + i
    masks, identup = make_host_constants(NB)
    in_maps = []
    def wpack(w):        # [512, 64] -> SBUF layout [128, 256] (d-tile on free dim)
        return np.ascontiguousarray(
            w.reshape(4, 128, DH).transpose(1, 0, 2).reshape(128, 4 * DH)).astype(bf)

    for h in range(N_CORES):
        s = slice(DH * h, DH * (h + 1))
        in_maps.append({
            "xT": xT,
            "wq": wpack(Wq[:, s] / 64.0),
            "wk": wpack(Wkv[:, DH * h:DH * (h + 1)]),
            "wv": wpack(Wkv[:, DIM + DH * h:DIM + DH * (h + 1)]),
            "wo": np.ascontiguousarray(Wo[s, :]).astype(bf),
            "masks": masks,
            "identup": identup,
        })
    return in_maps


def kernel(x, Wq, Wkv, Wo, bo, _run_kwargs=None):
    from concourse.bass_utils import run_bass_kernel_spmd
    x = np.asarray(x, np.float32)
    NB = x.shape[1]
    nc = _get_compiled(NB)
    in_maps = make_in_maps(np.asarray(x), np.asarray(Wq), np.asarray(Wkv), np.asarray(Wo), NB)
    res = run_bass_kernel_spmd(nc, in_maps, core_ids=list(range(N_CORES)),
                               **(_run_kwargs or {}))
    oT = np.zeros((DIM, x.shape[0] * NB), np.float32)
    for c in range(N_CORES):
        oT += res.results[c]["oT"].astype(np.float32)
    # invert chunk-paired layout: col = 1024c + 512b + i  ->  [b, n, D]
    out = (oT.reshape(DIM, NB // CH, 2, CH).transpose(2, 1, 3, 0)
           .reshape(x.shape[0], NB, DIM).astype(np.float32) + np.asarray(bo, np.float32))
    if _run_kwargs is not None:
        _CACHE["last_results"] = res
    return out



# revision 16
# speedup vs baseline: 1.0098x; 1.0098x over previous
"""Trainium2 Bass kernel for nn_Attention_49168785605257.

Causal multi-head self-attention: B=2, N=4096, DIM=512, H=8, DH=64.
Reference applies dim_head**-0.5 scaling TWICE; folded here into the exp's
scale parameter (1/64) together with the 16x q/k weight pre-scales
(1/16384 net).

Sharding: one head per NeuronCore (8 cores). Each core computes its head's
attention for BOTH batches (packed into partition halves 0-63 / 64-127) and
its partial output projection o_h = attn_out_h @ Wo[64h:64h+64, :].  The host
sums the 8 partials and adds the bias.

Device-side formulation (per core):
  - All tensors carried transposed ([feature, token]); host pre-transposes x.
  - q and k projections fused into ONE fp8e4 DoubleRow matmul chain per
    batch-chunk (stationary [Wq|Wk] is 128 wide, two 256-deep k-tile pairs,
    0.5 cyc/row); the fp32 PSUM is cast to bf16 and DMA'd into qT/kT (DMA
    shifts partitions; engines cannot).  DoubleRow only works on full-PE
    (128,128) tiles at position (0,0), so S^T stays bf16 on quadrant tiles.
  - v projection and A@V stay bf16 by default (FP8_AV flips A@V to fp8e4
    DoubleRow over paired j-blocks when precision allows).
  - Flash-attention in S^T orientation: exp on ScalarE with scale=1/16384
    (PSUM->SBUF, [128,1024] groups covering both batches); causal masking by
    multiplying the exp output of the diagonal j-blocks with 0/1 masks; A@V
    accumulated in PSUM with v augmented by a ones-column so row 64 collects
    the softmax denominators; 1/den via DVE reciprocal; denominator broadcast
    across partitions on GPSIMD; normalize late (softmax linearity).
"""

import os
import sys
from contextlib import ExitStack

import numpy as np

for _p in ("/opt/trn_rl_repo", "/root/.axon_site/_ro/trn_rl_repo"):
    if _p not in sys.path and os.path.isdir(_p):
        sys.path.append(_p)

import ml_dtypes  # noqa: E402

B, N, DIM, H, DH = 2, 4096, 512, 8, 64
N_CORES = 8
CH = 512            # i-chunk width (tokens)
JB = 128            # j-block width (tokens)
EXP_SCALE = 1.0 / 16384.0   # 1/64 (double dim_head**-0.5) * 1/256 (16x Wq,Wk)
FP8_AV = False      # fp8e4 DoubleRow A@V over paired sub-diagonal j-blocks
DR_PROJ = False     # fp8e4 DoubleRow fused q+k projection (else bf16 baseline)

BF16 = "bfloat16"
F32 = "float32"


def _pin_act_tables():
    """Make Exp resolve only to one table set so the kernel never swaps ACT
    table sets mid-run. Best-effort."""
    try:
        import concourse.bacc as bacc
        import concourse.hw_specs as hw_specs
        import concourse.mybir as mybir
        orig = hw_specs.get_activation_tables

        def patched(module_arch):
            try:
                tabs = dict(orig(module_arch))
                both = {mybir.ActivationFunctionType.Exp, mybir.ActivationFunctionType.Ln}
                target = None
                for name, funcs in tabs.items():
                    if both <= funcs:
                        target = name
                        break
                if target is None:
                    return tabs
                out = {}
                for name, funcs in tabs.items():
                    out[name] = set(funcs) if name == target else set(funcs) - both
                return out
            except Exception:
                return orig(module_arch)

        bacc.get_activation_tables = patched
    except Exception:
        pass


def build_attention_kernel(nc, NB: int):
    """Emit the per-core program. NB = tokens per batch (4096 full size)."""
    import concourse.mybir as mybir
    import concourse.tile as tile
    _pin_act_tables()

    bf16 = mybir.dt.bfloat16
    f32 = mybir.dt.float32
    fp8 = mybir.dt.float8e4
    mult = mybir.AluOpType.mult
    Exp = mybir.ActivationFunctionType.Exp
    DR = mybir.MatmulPerfMode.DoubleRow

    NCH = NB // CH          # i-chunks per batch
    JTB = NB // JB          # j-blocks per batch

    xT_d = nc.dram_tensor("xT", [DIM, 2 * NB], bf16, kind="ExternalInput").ap()
    x8_d = nc.dram_tensor("x8", [128, 8 * NB], fp8, kind="ExternalInput").ap()
    wqk8_d = nc.dram_tensor("wqk8", [128, 512], fp8, kind="ExternalInput").ap()
    wqb_d = nc.dram_tensor("wqb", [128, 4 * DH], bf16, kind="ExternalInput").ap()
    wkb_d = nc.dram_tensor("wkb", [128, 4 * DH], bf16, kind="ExternalInput").ap()
    wv_d = nc.dram_tensor("wv", [128, 4 * DH], bf16, kind="ExternalInput").ap()
    wo_d = nc.dram_tensor("wo", [DH, DIM], bf16, kind="ExternalInput").ap()
    mask_d = nc.dram_tensor("masks", [128, 256], bf16, kind="ExternalInput").ap()
    idup_d = nc.dram_tensor("identup", [128, DH], bf16, kind="ExternalInput").ap()
    oT_d = nc.dram_tensor("oT", [DIM, 2 * NB], bf16, kind="ExternalOutput").ap()

    with tile.TileContext(nc) as tc, ExitStack() as ctx:
        const = ctx.enter_context(tc.tile_pool(name="const", bufs=1))
        xpool = ctx.enter_context(tc.tile_pool(name="xp", bufs=12))
        x8pool = ctx.enter_context(tc.tile_pool(name="x8p", bufs=6))
        big = ctx.enter_context(tc.tile_pool(name="big", bufs=1))
        ptp = ctx.enter_context(tc.tile_pool(name="ptp", bufs=8))
        q8p = ctx.enter_context(tc.tile_pool(name="q8p", bufs=3))
        rp = ctx.enter_context(tc.tile_pool(name="rp", bufs=3))
        op_sb_pool = ctx.enter_context(tc.tile_pool(name="osb", bufs=4))
        ps_pool = ctx.enter_context(tc.tile_pool(name="ps", bufs=2, space="PSUM"))
        av_pool = ctx.enter_context(tc.tile_pool(name="av", bufs=1, space="PSUM"))
        pv_pool = ctx.enter_context(tc.tile_pool(name="pv", bufs=2, space="PSUM"))

        # ---- weights first so chunk-0 projections can start ASAP ----
        wqk8_sb = const.tile([128, 512], fp8, tag="wqk8")
        wqb_sb = const.tile([128, 4 * DH], bf16, tag="wqb")
        wkb_sb = const.tile([128, 4 * DH], bf16, tag="wkb")
        wv_sb = const.tile([128, 4 * DH], bf16, tag="wv")
        if DR_PROJ:
            nc.sync.dma_start(wqk8_sb[:], wqk8_d[:, :])
        else:
            nc.sync.dma_start(wqb_sb[:], wqb_d[:, :])
            nc.sync.dma_start(wkb_sb[:], wkb_d[:, :])
        nc.sync.dma_start(wv_sb[:], wv_d[:, :])
        wo_sb = const.tile([DH, DIM], bf16, tag="wo")
        mask_sb = const.tile([128, 256], bf16, tag="mask")
        idup_sb = const.tile([128, DH], bf16, tag="idup")

        # ---- persistent activations (partition halves: rows 0-63 batch0, 64-127 batch1) ----
        qT = big.tile([128, NB], bf16, tag="qT")
        kT = big.tile([128, NB], bf16, tag="kT")
        vT = big.tile([128, NB], bf16, tag="vT")
        vaug = [big.tile([128, 65 * JTB], bf16, tag=f"vaug{b}", name=f"vaug{b}")
                for b in range(2)]

        def r3(t2, t=2):
            return t2.rearrange("p (t n) -> p t n", t=t)

        xts_pend = {}
        x8s_pend = {}

        def emit_xt(c):
            """Issue the x-chunk DMAs for chunk c (early, to dodge sync-queue
            head-of-line blocking behind epilogue output DMAs)."""
            xts = []
            for d in range(4):
                xt = xpool.tile([128, 1024], bf16, tag="xt", name=f"xt{c}_{d}")
                nc.gpsimd.dma_start(xt[:], xT_d[128 * d:128 * (d + 1), 1024 * c:1024 * (c + 1)])
                xts.append(xt)
            xts_pend[c] = xts
            if not DR_PROJ:
                return
            x8s = []
            x8r = x8_d.rearrange("p (pr t n) -> p pr t n", pr=2, t=2)
            for pr in range(2):
                x8t = x8pool.tile([128, 2048], fp8, tag="x8t", name=f"x8t{c}_{pr}")
                nc.gpsimd.dma_start(r3(x8t)[:, :, :],
                                    x8r[:, pr, :, 1024 * c:1024 * (c + 1)])
                x8s.append(x8t)
            x8s_pend[c] = x8s

        def emit_chunk_prep(c, ws=(0, 1, 2)):
            """Fused q+k fp8 DoubleRow projection (+ bf16 repack via DMA) and
            bf16 v projection (+ v transposes) for chunk c."""
            i0 = CH * c
            if c == 0 and 0 in ws:
                nc.sync.dma_start(idup_sb[:], idup_d[:, :])
            if DR_PROJ and (0 in ws or 1 in ws):
                x8s = x8s_pend[c]
                for b in (([0] if 0 in ws else []) + ([1] if 1 in ws else [])):
                    ps = pv_pool.tile([128, CH], f32, tag="pv", name=f"qk{b}")
                    for pr in range(2):
                        nc.tensor.matmul(
                            ps[:, :],
                            r3(wqk8_sb[:, 256 * pr:256 * (pr + 1)])[:, :, :],
                            r3(x8s[pr])[:, :, 512 * b:512 * (b + 1)],
                            start=(pr == 0), stop=(pr == 1), perf_mode=DR,
                            skip_group_check=True)
                    stg = q8p.tile([128, CH], bf16, tag="stg", name=f"stg{b}")
                    nc.vector.tensor_copy(stg[:], ps[:, :])
                    nc.gpsimd.dma_start(qT[64 * b:64 * (b + 1), i0:i0 + CH], stg[0:64, :])
                    nc.gpsimd.dma_start(kT[64 * b:64 * (b + 1), i0:i0 + CH], stg[64:128, :])
            elif 0 in ws or 1 in ws:
                xts = xts_pend[c]
                for w_sb, dst in (((wqb_sb, qT),) if 0 in ws else ()) + \
                                 (((wkb_sb, kT),) if 1 in ws else ()):
                    ps = pv_pool.tile([128, CH], f32, tag="pv", name="qkb")
                    for d in range(4):
                        nc.tensor.matmul(ps[0:64, :], w_sb[:, d * DH:(d + 1) * DH],
                                         xts[d][:, 0:512],
                                         start=(d == 0), stop=(d == 3), tile_position=(0, 0),
                                         skip_group_check=True)
                        nc.tensor.matmul(ps[64:128, :], w_sb[:, d * DH:(d + 1) * DH],
                                         xts[d][:, 512:1024],
                                         start=(d == 0), stop=(d == 3), tile_position=(0, 64),
                                         skip_group_check=True)
                    nc.vector.tensor_copy(dst[:, i0:i0 + CH], ps[:, :])
            if c == 0 and 2 in ws:
                nc.sync.dma_start(mask_sb[:], mask_d[:, :])
                nc.sync.dma_start(wo_sb[:], wo_d[:, :])
            if 2 not in ws:
                return
            # --- v projection (bf16) ---
            xts = xts_pend.pop(c)
            x8s_pend.pop(c, None)
            psv = pv_pool.tile([128, CH], f32, tag="pv", name="vproj")
            for d in range(4):
                nc.tensor.matmul(psv[0:64, :], wv_sb[:, d * DH:(d + 1) * DH], xts[d][:, 0:512],
                                 start=(d == 0), stop=(d == 3), tile_position=(0, 0),
                                 skip_group_check=True)
                nc.tensor.matmul(psv[64:128, :], wv_sb[:, d * DH:(d + 1) * DH], xts[d][:, 512:1024],
                                 start=(d == 0), stop=(d == 3), tile_position=(0, 64),
                                 skip_group_check=True)
            nc.vector.tensor_copy(vT[:, i0:i0 + CH], psv[:, :])
            # --- v transposes into vaug (ones-column augmented) ---
            pstt = pv_pool.tile([128, 512], bf16, tag="pv", name="pstt")
            for k, tt in enumerate(range(4 * c, 4 * c + 4)):
                nc.tensor.matmul(pstt[:, 128 * k:128 * k + 64],
                                 vT[0:64, JB * tt:JB * (tt + 1)], idup_sb[0:64, :],
                                 is_transpose=True, tile_position=(0, 0), skip_group_check=True)
                nc.tensor.matmul(pstt[:, 128 * k + 64:128 * (k + 1)],
                                 vT[64:128, JB * tt:JB * (tt + 1)], idup_sb[64:128, :],
                                 is_transpose=True, tile_position=(64, 0), skip_group_check=True)
            tt0 = 4 * c
            for b in range(2):
                dst = vaug[b][:, 65 * tt0:65 * (tt0 + 4)].rearrange(
                    "p (four x) -> p four x", four=4)[:, :, 0:64]
                src = pstt[:].rearrange("p (four two d) -> p four two d",
                                        four=4, two=2)[:, :, b, :]
                nc.vector.tensor_copy(dst, src)

        def emit_epilogue_a2(c, outT_un):
            """1/den chain, off the attention critical path.  DVE reciprocal
            (same-partition: engines cannot shift partitions) keeps ScalarE —
            the exp bottleneck — free; the DMA bounce moves the row to
            partition 0 for the GPSIMD broadcast."""
            denp0 = rp.tile([1, 1024], f32, tag="denp0")
            nc.gpsimd.dma_start(denp0[0:1, :], outT_un[64:65, 0:1024])
            recip = rp.tile([1, 1024], f32, tag="recip")
            nc.vector.reciprocal_approx_fast(recip[0:1, :], denp0[0:1, :])
            recipb = rp.tile([64, 1024], f32, tag="recipb")
            nc.gpsimd.partition_broadcast(recipb[0:64, :], recip[0:1, :], channels=64)
            return recipb

        def emit_epilogue_b(c, outT_un, recipb, dblks=range(4), outTn=None):
            """Deferred per-chunk tail: normalize and project; stream out."""
            if outTn is None:
                outTn = rp.tile([64, 1024], bf16, tag="outTn")
                nc.vector.tensor_tensor(outTn[:], outT_un[0:64, 0:1024], recipb[:], mult)
            for dblk in dblks:
                o_sb = op_sb_pool.tile([128, 1024], bf16, tag="o")
                for b in range(2):
                    opp = pv_pool.tile([128, 512], f32, tag="pv", name=f"opp{b}")
                    nc.tensor.matmul(opp[:], wo_sb[:, 128 * dblk:128 * (dblk + 1)],
                                     outTn[0:64, 512 * b:512 * b + 512],
                                     skip_group_check=True)
                    nc.vector.tensor_copy(o_sb[:, 512 * b:512 * (b + 1)], opp[:])
                nc.sync.dma_start(oT_d[128 * dblk:128 * (dblk + 1), 1024 * c:1024 * (c + 1)],
                                  o_sb[:])
            return outTn

        def emit_s(c, jb, off):
            """S^T matmuls for block (c, jb) -> fresh PSUM tile."""
            i0 = CH * c
            pss = ps_pool.tile([128, 1024], f32, tag="s")
            nc.tensor.matmul(pss[:, off:512], kT[0:64, JB * jb:JB * (jb + 1)],
                             qT[0:64, i0 + off:i0 + CH],
                             start=True, stop=True, tile_position=(0, 0), skip_group_check=True)
            nc.tensor.matmul(pss[:, 512 + off:1024], kT[64:128, JB * jb:JB * (jb + 1)],
                             qT[64:128, i0 + off:i0 + CH],
                             start=True, stop=True, tile_position=(64, 0), skip_group_check=True)
            return pss

        n_up = min(2, NCH)          # chunks prepped upfront (short early chunks)
        emit_xt(0)
        nc.gpsimd.memset(vaug[0][:], 1.0)
        nc.gpsimd.memset(vaug[1][:], 1.0)
        for cc in range(1, n_up):
            emit_xt(cc)
        for cc in range(n_up):
            emit_chunk_prep(cc)
        if n_up < NCH:
            emit_xt(n_up)
        pending_b = None
        for c in range(NCH):
            i0 = CH * c
            # ---- attention for i-chunk c (prev tail + next prep interleaved) ----
            pso = av_pool.tile([65, 1024], f32, tag="av")
            njb = 4 * (c + 1)
            mid = max(1, njb // 2)
            pair_pend = None     # (pt2, pp) half-filled fp8 pair
            for jb in range(njb):
                if jb == 2 and pending_b is not None:
                    pending_b = pending_b[:3] + (emit_epilogue_b(*pending_b, dblks=(0, 1)),)
                    if n_up <= c + 2 < NCH:
                        emit_xt(c + 2)
                if jb == 4 and pending_b is not None and len(pending_b) == 4:
                    emit_epilogue_b(*pending_b[:3], dblks=(2, 3), outTn=pending_b[3])
                    pending_b = None
                if jb == mid and n_up <= c + 1 < NCH:
                    emit_chunk_prep(c + 1, ws=(0, 1))
                if jb == mid + 2 and n_up <= c + 1 < NCH:
                    emit_chunk_prep(c + 1, ws=(2,))
                # diagonal blocks: only i-columns >= 128t are causally valid
                t = jb - 4 * c
                if FP8_AV and t < 0:
                    # fp8 DoubleRow A@V over paired sub-diagonal j-blocks
                    pss = emit_s(c, jb, 0)
                    if pair_pend is None:
                        pt2 = ptp.tile([128, 2048], fp8, tag="pt8", name="pt8")
                        nc.scalar.activation(r3(pt2)[:, 0, :], pss[:, :], Exp,
                                             scale=EXP_SCALE)
                        pair_pend = pt2
                        continue
                    pt2 = pair_pend
                    pair_pend = None
                    nc.scalar.activation(r3(pt2)[:, 1, :], pss[:, :], Exp,
                                         scale=EXP_SCALE)
                    pp = jb // 2
                    for b in range(2):
                        nc.tensor.matmul(
                            pso[:, 512 * b:512 * (b + 1)],
                            r3(vaug[b][:, 130 * pp:130 * (pp + 1)], t=2),
                            r3(pt2)[:, :, 512 * b:512 * (b + 1)],
                            start=(jb == 1), stop=False, perf_mode=DR,
                            skip_group_check=True)
                    continue
                off = 128 * t if t > 0 else 0
                pss = emit_s(c, jb, off)
                pt = ptp.tile([128, 1024], bf16, tag="pt")
                if off:
                    sub = lambda ap: ap.rearrange("p (h w) -> p h w", h=2)[:, :, off:]
                    nc.scalar.activation(sub(pt[:]), sub(pss[:]), Exp, scale=EXP_SCALE)
                else:
                    nc.scalar.activation(pt[:], pss[:], Exp, scale=EXP_SCALE)
                if t >= 0:
                    # only the 128-col diagonal square is partially masked
                    msub = pt[:].rearrange("p (h w) -> p h w", h=2)[:, :, off:off + 128]
                    nc.vector.tensor_tensor(
                        msub, msub,
                        mask_sb[:, 0:256].rearrange("p (h w) -> p h w", h=2), mult)
                first = (jb == 0)
                nc.tensor.matmul(pso[:, off:512], vaug[0][:, 65 * jb:65 * jb + 65], pt[:, off:512],
                                 start=first, stop=(jb == njb - 1), skip_group_check=True)
                nc.tensor.matmul(pso[:, 512 + off:1024], vaug[1][:, 65 * jb:65 * jb + 65],
                                 pt[:, 512 + off:1024],
                                 start=first, stop=(jb == njb - 1), skip_group_check=True)

            # ---- epilogue part A: evacuate pso in one fp32 copy ----
            outT_un = rp.tile([65, 1024], f32, tag="outT_un")
            nc.vector.tensor_copy(outT_un[:], pso[0:65, 0:1024])
            if (c + 2) < NCH and (c + 2) not in xts_pend and n_up <= c + 2:
                emit_xt(c + 2)   # fallback if the jb==2 site did not fire
            pending_b = (c, outT_un, emit_epilogue_a2(c, outT_un))
        if pending_b is not None:
            emit_epilogue_b(*pending_b)
    return nc


def make_host_constants(NB: int):
    """0/1 masks for the diagonal j-block square and the stacked identity."""
    jj = np.arange(JB)[:, None]
    ii = np.arange(JB)[None, :]
    m = (ii >= jj).astype(np.float32)                    # [128, 128] diagonal square
    masks = np.concatenate([m, m], axis=1)               # [128, 256]
    identup = np.concatenate([np.eye(DH, dtype=np.float32)] * 2, axis=0)  # [128, 64]
    return (masks.astype(ml_dtypes.bfloat16), identup.astype(ml_dtypes.bfloat16))


_CACHE = {}


def _get_compiled(NB: int):
    key = ("nc", NB)
    if key not in _CACHE:
        import concourse.bacc as bacc
        nc = bacc.Bacc("TRN2", debug=False, num_devices=N_CORES)
        build_attention_kernel(nc, NB)
        nc.compile()
        _CACHE[key] = nc
    return _CACHE[key]


def make_in_maps(x, Wq, Wkv, Wo, NB: int):
    bf = ml_dtypes.bfloat16
    f8 = ml_dtypes.float8_e4m3
    NB = x.shape[1]
    nb_total = x.shape[0] * NB
    xT = x.reshape(nb_total, DIM).T            # [512, B*NB], batch-major cols
    xT = xT.reshape(DIM, 2, NB // CH, CH).transpose(0, 2, 1, 3).reshape(DIM, nb_total)
    xT = np.ascontiguousarray(xT)              # chunk-paired: col = 1024c + 512b + i
    # fp8 copy in paired-d-tile layout: [128, pair(2), t(2), 2NB]
    x8 = xT.reshape(2, 2, 128, nb_total).transpose(2, 0, 1, 3).reshape(128, 4 * nb_total)
    x8 = np.ascontiguousarray(x8).astype(f8)
    xT_bf = xT.astype(bf)
    masks, identup = make_host_constants(NB)
    in_maps = []

    def wqk8pack(wq, wk):
        # [512, 64]x2 -> [128, 512] fp8: col = pair*256 + t*128 + (q|k index)
        wqk = np.concatenate([wq, wk], axis=1)           # [512, 128]
        return np.ascontiguousarray(
            wqk.reshape(2, 2, 128, 128).transpose(2, 0, 1, 3).reshape(128, 512)
        ).astype(f8)

    def wpack(w):        # [512, 64] -> SBUF layout [128, 256] (d-tile on free dim)
        return np.ascontiguousarray(
            w.reshape(4, 128, DH).transpose(1, 0, 2).reshape(128, 4 * DH)).astype(bf)

    for h in range(N_CORES):
        s = slice(DH * h, DH * (h + 1))
        in_maps.append({
            "xT": xT_bf,
            "x8": x8,
            "wqk8": wqk8pack(Wq[:, s] * 16.0, Wkv[:, DH * h:DH * (h + 1)] * 16.0),
            "wqb": wpack(Wq[:, s] * 16.0),
            "wkb": wpack(Wkv[:, DH * h:DH * (h + 1)] * 16.0),
            "wv": wpack(Wkv[:, DIM + DH * h:DIM + DH * (h + 1)]),
            "wo": np.ascontiguousarray(Wo[s, :]).astype(bf),
            "masks": masks,
            "identup": identup,
        })
    return in_maps


def kernel(x, Wq, Wkv, Wo, bo, _run_kwargs=None):
    from concourse.bass_utils import run_bass_kernel_spmd
    x = np.asarray(x, np.float32)
    NB = x.shape[1]
    nc = _get_compiled(NB)
    in_maps = make_in_maps(np.asarray(x), np.asarray(Wq), np.asarray(Wkv), np.asarray(Wo), NB)
    res = run_bass_kernel_spmd(nc, in_maps, core_ids=list(range(N_CORES)),
                               **(_run_kwargs or {}))
    oT = np.zeros((DIM, x.shape[0] * NB), np.float32)
    for c in range(N_CORES):
        oT += res.results[c]["oT"].astype(np.float32)
    # invert chunk-paired layout: col = 1024c + 512b + i  ->  [b, n, D]
    out = (oT.reshape(DIM, NB // CH, 2, CH).transpose(2, 1, 3, 0)
           .reshape(x.shape[0], NB, DIM).astype(np.float32) + np.asarray(bo, np.float32))
    if _run_kwargs is not None:
        _CACHE["last_results"] = res
    return out


# revision 19
# speedup vs baseline: 1.0174x; 1.0075x over previous
"""Trainium2 Bass kernel for nn_Attention_49168785605257.

Causal multi-head self-attention: B=2, N=4096, DIM=512, H=8, DH=64.
Reference applies dim_head**-0.5 scaling TWICE; folded here into the exp's
scale parameter (1/64) together with the 16x q/k weight pre-scales
(1/16384 net).

Sharding: one head per NeuronCore (8 cores). Each core computes its head's
attention for BOTH batches (packed into partition halves 0-63 / 64-127) and
its partial output projection o_h = attn_out_h @ Wo[64h:64h+64, :].  The host
sums the 8 partials and adds the bias.

Device-side formulation (per core):
  - All tensors carried transposed ([feature, token]); host pre-transposes x.
  - q and k projections fused into ONE fp8e4 DoubleRow matmul chain per
    batch-chunk (stationary [Wq|Wk] is 128 wide, two 256-deep k-tile pairs,
    0.5 cyc/row); the fp32 PSUM is cast to bf16 and DMA'd into qT/kT (DMA
    shifts partitions; engines cannot).  DoubleRow only works on full-PE
    (128,128) tiles at position (0,0), so S^T stays bf16 on quadrant tiles.
  - v projection and A@V stay bf16 by default (FP8_AV flips A@V to fp8e4
    DoubleRow over paired j-blocks when precision allows).
  - Flash-attention in S^T orientation: exp on ScalarE with scale=1/16384
    (PSUM->SBUF, [128,1024] groups covering both batches); causal masking by
    multiplying the exp output of the diagonal j-blocks with 0/1 masks; A@V
    accumulated in PSUM with v augmented by a ones-column so row 64 collects
    the softmax denominators; 1/den via DVE reciprocal; denominator broadcast
    across partitions on GPSIMD; normalize late (softmax linearity).
"""

import os
import sys
from contextlib import ExitStack

import numpy as np

for _p in ("/opt/trn_rl_repo", "/root/.axon_site/_ro/trn_rl_repo"):
    if _p not in sys.path and os.path.isdir(_p):
        sys.path.append(_p)

import ml_dtypes  # noqa: E402

B, N, DIM, H, DH = 2, 4096, 512, 8, 64
N_CORES = 8
CH = 512            # i-chunk width (tokens)
JB = 128            # j-block width (tokens)
EXP_SCALE = 1.0 / 16384.0   # 1/64 (double dim_head**-0.5) * 1/256 (16x Wq,Wk)
FP8_AV = False      # fp8e4 DoubleRow A@V over paired sub-diagonal j-blocks
DR_PROJ = False     # fp8e4 DoubleRow fused q+k projection (else bf16 baseline)

BF16 = "bfloat16"
F32 = "float32"


def _pin_act_tables():
    """Make Exp resolve only to one table set so the kernel never swaps ACT
    table sets mid-run. Best-effort."""
    try:
        import concourse.bacc as bacc
        import concourse.hw_specs as hw_specs
        import concourse.mybir as mybir
        orig = hw_specs.get_activation_tables

        def patched(module_arch):
            try:
                tabs = dict(orig(module_arch))
                both = {mybir.ActivationFunctionType.Exp, mybir.ActivationFunctionType.Ln}
                target = None
                for name, funcs in tabs.items():
                    if both <= funcs:
                        target = name
                        break
                if target is None:
                    return tabs
                out = {}
                for name, funcs in tabs.items():
                    out[name] = set(funcs) if name == target else set(funcs) - both
                return out
            except Exception:
                return orig(module_arch)

        bacc.get_activation_tables = patched
    except Exception:
        pass


def build_attention_kernel(nc, NB: int):
    """Emit the per-core program. NB = tokens per batch (4096 full size)."""
    import concourse.mybir as mybir
    import concourse.tile as tile
    _pin_act_tables()

    bf16 = mybir.dt.bfloat16
    f32 = mybir.dt.float32
    fp8 = mybir.dt.float8e4
    mult = mybir.AluOpType.mult
    Exp = mybir.ActivationFunctionType.Exp
    DR = mybir.MatmulPerfMode.DoubleRow

    NCH = NB // CH          # i-chunks per batch
    JTB = NB // JB          # j-blocks per batch

    xT_d = nc.dram_tensor("xT", [DIM, 2 * NB], bf16, kind="ExternalInput").ap()
    if DR_PROJ:
        x8_d = nc.dram_tensor("x8", [128, 8 * NB], fp8, kind="ExternalInput").ap()
        wqk8_d = nc.dram_tensor("wqk8", [128, 512], fp8, kind="ExternalInput").ap()
    wqb_d = nc.dram_tensor("wqb", [128, 4 * DH], bf16, kind="ExternalInput").ap()
    wkb_d = nc.dram_tensor("wkb", [128, 4 * DH], bf16, kind="ExternalInput").ap()
    wv_d = nc.dram_tensor("wv", [128, 4 * DH], bf16, kind="ExternalInput").ap()
    wo_d = nc.dram_tensor("wo", [DH, DIM], bf16, kind="ExternalInput").ap()
    mask_d = nc.dram_tensor("masks", [128, 256], bf16, kind="ExternalInput").ap()
    idup_d = nc.dram_tensor("identup", [128, DH], bf16, kind="ExternalInput").ap()
    oT_d = nc.dram_tensor("oT", [DIM, 2 * NB], bf16, kind="ExternalOutput").ap()

    with tile.TileContext(nc) as tc, ExitStack() as ctx:
        const = ctx.enter_context(tc.tile_pool(name="const", bufs=1))
        xpool = ctx.enter_context(tc.tile_pool(name="xp", bufs=12))
        x8pool = ctx.enter_context(tc.tile_pool(name="x8p", bufs=6))
        big = ctx.enter_context(tc.tile_pool(name="big", bufs=1))
        ptp = ctx.enter_context(tc.tile_pool(name="ptp", bufs=8))
        q8p = ctx.enter_context(tc.tile_pool(name="q8p", bufs=3))
        rp = ctx.enter_context(tc.tile_pool(name="rp", bufs=3))
        op_sb_pool = ctx.enter_context(tc.tile_pool(name="osb", bufs=4))
        ps_pool = ctx.enter_context(tc.tile_pool(name="ps", bufs=2, space="PSUM"))
        av_pool = ctx.enter_context(tc.tile_pool(name="av", bufs=1, space="PSUM"))
        pv_pool = ctx.enter_context(tc.tile_pool(name="pv", bufs=2, space="PSUM"))

        # ---- weights first so chunk-0 projections can start ASAP ----
        if DR_PROJ:
            wqk8_sb = const.tile([128, 512], fp8, tag="wqk8")
        wqb_sb = const.tile([128, 4 * DH], bf16, tag="wqb")
        wkb_sb = const.tile([128, 4 * DH], bf16, tag="wkb")
        wv_sb = const.tile([128, 4 * DH], bf16, tag="wv")
        if DR_PROJ:
            nc.sync.dma_start(wqk8_sb[:], wqk8_d[:, :])
        else:
            nc.sync.dma_start(wqb_sb[:], wqb_d[:, :])
            nc.sync.dma_start(wkb_sb[:], wkb_d[:, :])
        nc.sync.dma_start(wv_sb[:], wv_d[:, :])
        wo_sb = const.tile([DH, DIM], bf16, tag="wo")
        mask_sb = const.tile([128, 256], bf16, tag="mask")
        idup_sb = const.tile([128, DH], bf16, tag="idup")

        # ---- persistent activations (partition halves: rows 0-63 batch0, 64-127 batch1) ----
        qT = big.tile([128, NB], bf16, tag="qT")
        kT = big.tile([128, NB], bf16, tag="kT")
        vT = big.tile([128, NB], bf16, tag="vT")
        vaug = [big.tile([128, 65 * JTB], bf16, tag=f"vaug{b}", name=f"vaug{b}")
                for b in range(2)]

        def r3(t2, t=2):
            return t2.rearrange("p (t n) -> p t n", t=t)

        xts_pend = {}
        x8s_pend = {}

        def emit_xt(c):
            """Issue the x-chunk DMAs for chunk c (early, to dodge sync-queue
            head-of-line blocking behind epilogue output DMAs)."""
            xts = []
            for d in range(4):
                xt = xpool.tile([128, 1024], bf16, tag="xt", name=f"xt{c}_{d}")
                nc.gpsimd.dma_start(xt[:], xT_d[128 * d:128 * (d + 1), 1024 * c:1024 * (c + 1)])
                xts.append(xt)
            xts_pend[c] = xts
            if not DR_PROJ:
                return
            x8s = []
            x8r = x8_d.rearrange("p (pr t n) -> p pr t n", pr=2, t=2)
            for pr in range(2):
                x8t = x8pool.tile([128, 2048], fp8, tag="x8t", name=f"x8t{c}_{pr}")
                nc.gpsimd.dma_start(r3(x8t)[:, :, :],
                                    x8r[:, pr, :, 1024 * c:1024 * (c + 1)])
                x8s.append(x8t)
            x8s_pend[c] = x8s

        def emit_chunk_prep(c, ws=(0, 1, 2)):
            """Fused q+k fp8 DoubleRow projection (+ bf16 repack via DMA) and
            bf16 v projection (+ v transposes) for chunk c."""
            i0 = CH * c
            if c == 0 and 0 in ws:
                nc.sync.dma_start(idup_sb[:], idup_d[:, :])
            if DR_PROJ and (0 in ws or 1 in ws):
                x8s = x8s_pend[c]
                for b in (([0] if 0 in ws else []) + ([1] if 1 in ws else [])):
                    ps = pv_pool.tile([128, CH], f32, tag="pv", name=f"qk{b}")
                    for pr in range(2):
                        nc.tensor.matmul(
                            ps[:, :],
                            r3(wqk8_sb[:, 256 * pr:256 * (pr + 1)])[:, :, :],
                            r3(x8s[pr])[:, :, 512 * b:512 * (b + 1)],
                            start=(pr == 0), stop=(pr == 1), perf_mode=DR,
                            skip_group_check=True)
                    stg = q8p.tile([128, CH], bf16, tag="stg", name=f"stg{b}")
                    nc.vector.tensor_copy(stg[:], ps[:, :])
                    nc.gpsimd.dma_start(qT[64 * b:64 * (b + 1), i0:i0 + CH], stg[0:64, :])
                    nc.gpsimd.dma_start(kT[64 * b:64 * (b + 1), i0:i0 + CH], stg[64:128, :])
            elif 0 in ws or 1 in ws:
                xts = xts_pend[c]
                for w_sb, dst in (((wqb_sb, qT),) if 0 in ws else ()) + \
                                 (((wkb_sb, kT),) if 1 in ws else ()):
                    ps = pv_pool.tile([128, CH], f32, tag="pv", name="qkb")
                    for d in range(4):
                        nc.tensor.matmul(ps[0:64, :], w_sb[:, d * DH:(d + 1) * DH],
                                         xts[d][:, 0:512],
                                         start=(d == 0), stop=(d == 3), tile_position=(0, 0),
                                         skip_group_check=True)
                        nc.tensor.matmul(ps[64:128, :], w_sb[:, d * DH:(d + 1) * DH],
                                         xts[d][:, 512:1024],
                                         start=(d == 0), stop=(d == 3), tile_position=(0, 64),
                                         skip_group_check=True)
                    nc.vector.tensor_copy(dst[:, i0:i0 + CH], ps[:, :])
            if c == 0 and 2 in ws:
                nc.sync.dma_start(mask_sb[:], mask_d[:, :])
                nc.sync.dma_start(wo_sb[:], wo_d[:, :])
            if 2 not in ws:
                return
            # --- v projection (bf16) ---
            xts = xts_pend.pop(c)
            x8s_pend.pop(c, None)
            psv = pv_pool.tile([128, CH], f32, tag="pv", name="vproj")
            for d in range(4):
                nc.tensor.matmul(psv[0:64, :], wv_sb[:, d * DH:(d + 1) * DH], xts[d][:, 0:512],
                                 start=(d == 0), stop=(d == 3), tile_position=(0, 0),
                                 skip_group_check=True)
                nc.tensor.matmul(psv[64:128, :], wv_sb[:, d * DH:(d + 1) * DH], xts[d][:, 512:1024],
                                 start=(d == 0), stop=(d == 3), tile_position=(0, 64),
                                 skip_group_check=True)
            nc.vector.tensor_copy(vT[:, i0:i0 + CH], psv[:, :])
            # --- v transposes into vaug (ones-column augmented) ---
            for tt in range(4 * c, 4 * c + 4):
                pst0 = pv_pool.tile([128, 64], bf16, tag="pv", name="pst0")
                pst1 = pv_pool.tile([128, 64], bf16, tag="pv", name="pst1")
                nc.tensor.matmul(pst0[:], vT[0:64, JB * tt:JB * (tt + 1)], idup_sb[0:64, :],
                                 is_transpose=True, tile_position=(0, 0), skip_group_check=True)
                nc.tensor.matmul(pst1[:], vT[64:128, JB * tt:JB * (tt + 1)], idup_sb[64:128, :],
                                 is_transpose=True, tile_position=(64, 0), skip_group_check=True)
                nc.vector.tensor_copy(vaug[0][:, 65 * tt:65 * tt + 64], pst0[:])
                nc.vector.tensor_copy(vaug[1][:, 65 * tt:65 * tt + 64], pst1[:])

        def emit_epilogue_a2(c, outT_un):
            """1/den chain, off the attention critical path.  DVE reciprocal
            (same-partition: engines cannot shift partitions) keeps ScalarE —
            the exp bottleneck — free; the DMA bounce moves the row to
            partition 0 for the GPSIMD broadcast."""
            denp0 = rp.tile([1, 1024], f32, tag="denp0")
            nc.gpsimd.dma_start(denp0[0:1, :], outT_un[64:65, 0:1024])
            recip = rp.tile([1, 1024], f32, tag="recip")
            nc.vector.reciprocal_approx_fast(recip[0:1, :], denp0[0:1, :])
            recipb = rp.tile([64, 1024], f32, tag="recipb")
            nc.gpsimd.partition_broadcast(recipb[0:64, :], recip[0:1, :], channels=64)
            return recipb

        def emit_epilogue_b(c, outT_un, recipb, dblks=range(4), outTn=None):
            """Deferred per-chunk tail: normalize and project; stream out."""
            if outTn is None:
                outTn = rp.tile([64, 1024], bf16, tag="outTn")
                nc.vector.tensor_tensor(outTn[:], outT_un[0:64, 0:1024], recipb[:], mult)
            for dblk in dblks:
                o_sb = op_sb_pool.tile([128, 1024], bf16, tag="o")
                for b in range(2):
                    opp = pv_pool.tile([128, 512], f32, tag="pv", name=f"opp{b}")
                    nc.tensor.matmul(opp[:], wo_sb[:, 128 * dblk:128 * (dblk + 1)],
                                     outTn[0:64, 512 * b:512 * b + 512],
                                     skip_group_check=True)
                    nc.vector.tensor_copy(o_sb[:, 512 * b:512 * (b + 1)], opp[:])
                nc.sync.dma_start(oT_d[128 * dblk:128 * (dblk + 1), 1024 * c:1024 * (c + 1)],
                                  o_sb[:])
            return outTn

        def emit_s(c, jb, off):
            """S^T matmuls for block (c, jb) -> fresh PSUM tile."""
            i0 = CH * c
            pss = ps_pool.tile([128, 1024], f32, tag="s")
            nc.tensor.matmul(pss[:, off:512], kT[0:64, JB * jb:JB * (jb + 1)],
                             qT[0:64, i0 + off:i0 + CH],
                             start=True, stop=True, tile_position=(0, 0), skip_group_check=True)
            nc.tensor.matmul(pss[:, 512 + off:1024], kT[64:128, JB * jb:JB * (jb + 1)],
                             qT[64:128, i0 + off:i0 + CH],
                             start=True, stop=True, tile_position=(64, 0), skip_group_check=True)
            return pss

        n_up = min(2, NCH)          # chunks prepped upfront (short early chunks)
        emit_xt(0)
        nc.gpsimd.memset(vaug[0][:], 1.0)
        nc.gpsimd.memset(vaug[1][:], 1.0)
        for cc in range(1, n_up):
            emit_xt(cc)
        for cc in range(n_up):
            emit_chunk_prep(cc)
        if n_up < NCH:
            emit_xt(n_up)
        pending_b = None
        for c in range(NCH):
            i0 = CH * c
            # ---- attention for i-chunk c (prev tail + next prep interleaved) ----
            pso = av_pool.tile([65, 1024], f32, tag="av")
            njb = 4 * (c + 1)
            mid = max(1, njb // 2)
            pair_pend = None     # (pt2, pp) half-filled fp8 pair
            for jb in range(njb):
                if jb == 2 and pending_b is not None:
                    pending_b = pending_b[:3] + (emit_epilogue_b(*pending_b, dblks=(0, 1)),)
                    if n_up <= c + 2 < NCH:
                        emit_xt(c + 2)
                if jb == 4 and pending_b is not None and len(pending_b) == 4:
                    emit_epilogue_b(*pending_b[:3], dblks=(2, 3), outTn=pending_b[3])
                    pending_b = None
                if jb == mid and n_up <= c + 1 < NCH:
                    emit_chunk_prep(c + 1, ws=(0, 1))
                if jb == mid + 2 and n_up <= c + 1 < NCH:
                    emit_chunk_prep(c + 1, ws=(2,))
                # diagonal blocks: only i-columns >= 128t are causally valid
                t = jb - 4 * c
                if FP8_AV and t < 0:
                    # fp8 DoubleRow A@V over paired sub-diagonal j-blocks
                    pss = emit_s(c, jb, 0)
                    if pair_pend is None:
                        pt2 = ptp.tile([128, 2048], fp8, tag="pt8", name="pt8")
                        nc.scalar.activation(r3(pt2)[:, 0, :], pss[:, :], Exp)
                        pair_pend = pt2
                        continue
                    pt2 = pair_pend
                    pair_pend = None
                    nc.scalar.activation(r3(pt2)[:, 1, :], pss[:, :], Exp)
                    pp = jb // 2
                    for b in range(2):
                        nc.tensor.matmul(
                            pso[:, 512 * b:512 * (b + 1)],
                            r3(vaug[b][:, 130 * pp:130 * (pp + 1)], t=2),
                            r3(pt2)[:, :, 512 * b:512 * (b + 1)],
                            start=(jb == 1), stop=False, perf_mode=DR,
                            skip_group_check=True)
                    continue
                off = 128 * t if t > 0 else 0
                pss = emit_s(c, jb, off)
                pt = ptp.tile([128, 1024], bf16, tag="pt")
                if off:
                    sub = lambda ap: ap.rearrange("p (h w) -> p h w", h=2)[:, :, off:]
                    nc.scalar.activation(sub(pt[:]), sub(pss[:]), Exp)
                else:
                    nc.scalar.activation(pt[:], pss[:], Exp)
                if t >= 0:
                    # only the 128-col diagonal square is partially masked
                    msub = pt[:].rearrange("p (h w) -> p h w", h=2)[:, :, off:off + 128]
                    nc.vector.tensor_tensor(
                        msub, msub,
                        mask_sb[:, 0:256].rearrange("p (h w) -> p h w", h=2), mult)
                first = (jb == 0)
                nc.tensor.matmul(pso[:, off:512], vaug[0][:, 65 * jb:65 * jb + 65], pt[:, off:512],
                                 start=first, stop=(jb == njb - 1), skip_group_check=True)
                nc.tensor.matmul(pso[:, 512 + off:1024], vaug[1][:, 65 * jb:65 * jb + 65],
                                 pt[:, 512 + off:1024],
                                 start=first, stop=(jb == njb - 1), skip_group_check=True)

            # ---- epilogue part A: evacuate pso in one fp32 copy ----
            outT_un = rp.tile([65, 1024], f32, tag="outT_un")
            nc.vector.tensor_copy(outT_un[:], pso[0:65, 0:1024])
            if (c + 2) < NCH and (c + 2) not in xts_pend and n_up <= c + 2:
                emit_xt(c + 2)   # fallback if the jb==2 site did not fire
            pending_b = (c, outT_un, emit_epilogue_a2(c, outT_un))
        if pending_b is not None:
            emit_epilogue_b(*pending_b)
    return nc


def make_host_constants(NB: int):
    """0/1 masks for the diagonal j-block square and the stacked identity."""
    jj = np.arange(JB)[:, None]
    ii = np.arange(JB)[None, :]
    m = (ii >= jj).astype(np.float32)                    # [128, 128] diagonal square
    masks = np.concatenate([m, m], axis=1)               # [128, 256]
    identup = np.concatenate([np.eye(DH, dtype=np.float32)] * 2, axis=0)  # [128, 64]
    return (masks.astype(ml_dtypes.bfloat16), identup.astype(ml_dtypes.bfloat16))


_CACHE = {}


def _get_compiled(NB: int):
    key = ("nc", NB)
    if key not in _CACHE:
        import concourse.bacc as bacc
        nc = bacc.Bacc("TRN2", debug=False, num_devices=N_CORES)
        build_attention_kernel(nc, NB)
        nc.compile()
        _CACHE[key] = nc
    return _CACHE[key]


def make_in_maps(x, Wq, Wkv, Wo, NB: int):
    bf = ml_dtypes.bfloat16
    f8 = ml_dtypes.float8_e4m3
    NB = x.shape[1]
    nb_total = x.shape[0] * NB
    xT = x.reshape(nb_total, DIM).T            # [512, B*NB], batch-major cols
    xT = xT.reshape(DIM, 2, NB // CH, CH).transpose(0, 2, 1, 3).reshape(DIM, nb_total)
    xT = np.ascontiguousarray(xT)              # chunk-paired: col = 1024c + 512b + i
    # fp8 copy in paired-d-tile layout: [128, pair(2), t(2), 2NB]
    x8 = xT.reshape(2, 2, 128, nb_total).transpose(2, 0, 1, 3).reshape(128, 4 * nb_total)
    x8 = np.ascontiguousarray(x8).astype(f8)
    xT_bf = xT.astype(bf)
    masks, identup = make_host_constants(NB)
    in_maps = []

    def wqk8pack(wq, wk):
        # [512, 64]x2 -> [128, 512] fp8: col = pair*256 + t*128 + (q|k index)
        wqk = np.concatenate([wq, wk], axis=1)           # [512, 128]
        return np.ascontiguousarray(
            wqk.reshape(2, 2, 128, 128).transpose(2, 0, 1, 3).reshape(128, 512)
        ).astype(f8)

    def wpack(w):        # [512, 64] -> SBUF layout [128, 256] (d-tile on free dim)
        return np.ascontiguousarray(
            w.reshape(4, 128, DH).transpose(1, 0, 2).reshape(128, 4 * DH)).astype(bf)

    for h in range(N_CORES):
        s = slice(DH * h, DH * (h + 1))
        im = {
            "xT": xT_bf,
            "wqb": wpack(Wq[:, s] / 64.0),
            "wkb": wpack(Wkv[:, DH * h:DH * (h + 1)]),
            "wv": wpack(Wkv[:, DIM + DH * h:DIM + DH * (h + 1)]),
            "wo": np.ascontiguousarray(Wo[s, :]).astype(bf),
            "masks": masks,
            "identup": identup,
        }
        if DR_PROJ:
            im["x8"] = x8
            im["wqk8"] = wqk8pack(Wq[:, s] * 16.0, Wkv[:, DH * h:DH * (h + 1)] * 16.0)
        in_maps.append(im)
    return in_maps


def kernel(x, Wq, Wkv, Wo, bo, _run_kwargs=None):
    from concourse.bass_utils import run_bass_kernel_spmd
    x = np.asarray(x, np.float32)
    NB = x.shape[1]
    nc = _get_compiled(NB)
    in_maps = make_in_maps(np.asarray(x), np.asarray(Wq), np.asarray(Wkv), np.asarray(Wo), NB)
    res = run_bass_kernel_spmd(nc, in_maps, core_ids=list(range(N_CORES)),
                               **(_run_kwargs or {}))
    oT = np.zeros((DIM, x.shape[0] * NB), np.float32)
    for c in range(N_CORES):
        oT += res.results[c]["oT"].astype(np.float32)
    # invert chunk-paired layout: col = 1024c + 512b + i  ->  [b, n, D]
    out = (oT.reshape(DIM, NB // CH, 2, CH).transpose(2, 1, 3, 0)
           .reshape(x.shape[0], NB, DIM).astype(np.float32) + np.asarray(bo, np.float32))
    if _run_kwargs is not None:
        _CACHE["last_results"] = res
    return out
